# revision 1
# baseline (speedup 1.0000x reference)
"""3D Haar wavelet transform (2x2x2, causal temporal pad) on 8 Trainium2 cores.

Input  x: (2, 3, 33, 512, 512) fp32
Output y: (2, 24, 17, 256, 256) fp32   (channel = 3*s + c, s = subband)

Sharding: pure data parallel over H — core ci handles input rows
[64*ci, 64*ci+64) i.e. output rows [32*ci, 32*ci+32).

Per-core algorithm (all shapes hardcoded):
  for (b, c, T'):  load tile A[128, 512] where partition p = i*64 + h
        (i = temporal offset in pair, h = input row), free = w.
     T'=0 uses the causal pad: both i-halves read frame 0.
  DVE: B[:, 0:256] = A[:, 0::2] + A[:, 1::2];  B[:, 256:512] = diff
     (the W-axis Haar stage; sw = k-subband bit in free dim)
  PE:  psum = W.T @ B with fixed 128x128 matrix W doing the T- and H-axis
     stages plus the partition permutation: out partition m = di*64+dj*32+q.
  ACT: staging[:, slot] = psum * 0.3536  (PSUM -> SBUF evacuation + scale)
  DMA out per T'-group g (sizes 9+8): for each (di,dj) 32-partition slice
     and sw half: [32, G, 256] -> y[b, 3*(4di+2dj+sw)+c, T'0:T'0+G, :, :].
"""

import numpy as np

import concourse.bacc as bacc
import concourse.mybir as mybir
from concourse import tile
from concourse.bass_utils import run_bass_kernel_spmd

P = 128
B_, C_, T_, H_, W_ = 2, 3, 33, 512, 512
NCORES = 8
HC = H_ // NCORES          # 64 input rows per core
TP = (T_ + 1) // 2         # 17 output frames
HP = HC // 2               # 32 output rows per core
WP = W_ // 2               # 256 output cols
SCALE = float(np.float32(0.3536))
F32 = mybir.dt.float32

# T' groups per (b, c): 17 = 9 + 8
T_GROUPS = [(0, 9), (9, 8)]


def _haar_matrix() -> np.ndarray:
    """W[p, m]: p = i*64 + h (h = 2q+j), m = di*64 + dj*32 + q, val (-1)^(i*di+j*dj)."""
    W = np.zeros((P, P), dtype=np.float32)
    for i in range(2):
        for h in range(HC):
            j = h & 1
            q = h >> 1
            for di in range(2):
                for dj in range(2):
                    m = di * 64 + dj * 32 + q
                    W[i * 64 + h, m] = (-1.0) ** (i * di + j * dj)
    return W


def build_nc():
    nc = bacc.Bacc("TRN2", target_bir_lowering=False, debug=False)
    x_d = nc.dram_tensor("x", [B_, C_, T_, HC, W_], F32, kind="ExternalInput")
    y_d = nc.dram_tensor("y", [B_, 8 * C_, TP, HP, WP], F32, kind="ExternalOutput")
    w_d = nc.inline_tensor(_haar_matrix(), name="haar_w")

    with tile.TileContext(nc) as tc:
        with (
            tc.tile_pool(name="wpool", bufs=1) as wpool,
            tc.tile_pool(name="apool", bufs=4) as apool,
            tc.tile_pool(name="bpool", bufs=4) as bpool,
            tc.tile_pool(name="stage", bufs=2) as stage_pool,
            tc.tile_pool(name="psum", bufs=8, space="PSUM") as psum_pool,
        ):
            w_sb = wpool.tile([P, P], F32)
            nc.sync.dma_start(out=w_sb[:], in_=w_d[:])

            for b in range(B_):
                for c in range(C_):
                    xbc = x_d[b, c]  # [33, 64, 512]
                    for t0, G in T_GROUPS:
                        cbig = stage_pool.tile([P, G * W_], F32)
                        for tg in range(G):
                            tp = t0 + tg
                            a = apool.tile([P, W_], F32)
                            if tp == 0:
                                # causal pad: both temporal slots read frame 0
                                f0 = xbc[0]  # [64, 512]
                                nc.sync.dma_start(out=a[0:64], in_=f0)
                                nc.sync.dma_start(out=a[64:128], in_=f0)
                            else:
                                src = xbc[2 * tp - 1 : 2 * tp + 1]  # [2, 64, 512]
                                nc.sync.dma_start(
                                    out=a[:],
                                    in_=src.rearrange("t h w -> (t h) w"),
                                )
                            # W-axis stage on DVE: sum/diff of adjacent w pairs
                            av = a.rearrange("p (w k) -> p k w", k=2)
                            bt = bpool.tile([P, W_], F32)
                            nc.vector.tensor_add(
                                out=bt[:, 0:WP], in0=av[:, 0], in1=av[:, 1]
                            )
                            nc.vector.tensor_sub(
                                out=bt[:, WP:W_], in0=av[:, 0], in1=av[:, 1]
                            )
                            # T+H stages as one matmul
                            ps = psum_pool.tile([P, W_], F32)
                            nc.tensor.matmul(
                                ps[:], w_sb[:], bt[:], start=True, stop=True
                            )
                            # evacuate + scale
                            nc.scalar.mul(
                                cbig[:, tg * W_ : (tg + 1) * W_], ps[:], SCALE
                            )
                        # grouped output DMAs
                        for m4 in range(4):  # (di, dj)
                            di, dj = m4 >> 1, m4 & 1
                            cv = cbig[m4 * 32 : (m4 + 1) * 32].rearrange(
                                "p (t sw w) -> p t sw w", t=G, sw=2
                            )
                            for sw in range(2):
                                s = 4 * di + 2 * dj + sw
                                dst = y_d[b, 3 * s + c, t0 : t0 + G].rearrange(
                                    "T h w -> h T w"
                                )
                                nc.sync.dma_start(out=dst, in_=cv[:, :, sw])
    nc.compile()
    return nc


_NC_CACHE = None


def _get_nc():
    global _NC_CACHE
    if _NC_CACHE is None:
        _NC_CACHE = build_nc()
    return _NC_CACHE


def kernel(x: np.ndarray) -> np.ndarray:
    assert x.shape == (B_, C_, T_, H_, W_), x.shape
    x = np.ascontiguousarray(x, dtype=np.float32)
    nc = _get_nc()
    in_maps = [
        {"x": np.ascontiguousarray(x[:, :, :, HC * ci : HC * (ci + 1), :])}
        for ci in range(NCORES)
    ]
    res = run_bass_kernel_spmd(nc, in_maps, core_ids=list(range(NCORES)))
    return np.concatenate([res.results[ci]["y"] for ci in range(NCORES)], axis=3)
